# revision 45
# baseline (speedup 1.0000x reference)
"""Deformable conv block (B=8, C=64, H=W=128, K=3) on 8 Trainium2 cores.

Data-parallel over batch: one sample per core. bf16 mask-and-shift bilinear
sampling pipeline with PE convs.

Structure (one core, one sample):
- xpad2 [128, 16896]: rows 0-63 hold x flat (y,x) with 2-row zero pads (data
  at 256); rows 64-127 hold x shifted one row earlier (data at 128) so one
  rhs window covers a ky pair at contraction 128.
- offset conv (18ch, 5x5): 6 PSUM-accumulated matmuls per 512-col block
  (ky pairs {0,1},{2,3} + single ky=4), out partitions = kx blocks at
  32-pitch (A: kx 0..2 -> [96,512], B: kx 3..4 -> [64,512]), stripe-major so
  weights load once per 4-block stripe. The kx column shifts are applied in
  the per-block dma transposes (shifted source window; wrapped edge
  partitions zeroed by small DMAs); DVE sums the 5 transposed blocks
  directly into offt [x, (y, d)] and clamps, per y-half.
- bilinear weight maps gg [x, (tap, corner, y)] = gy*gx built from offt with
  sign masks and edge masks, per y-half, all on DVE.
- sampling: 5 x-shifted transposed copies of x (xtc, partition = image x,
  free = (c, y)); per (tap, y-half): 9 corner products + 8 adds
  (tensor_tensor, gg broadcast over c via stride-0), 3 of 18 units run on
  the Pool engine. Act reorders acc (c,y)->(y,c) into a pitch-128 ayc (the
  HW 3D dma transpose reads f = m*128 + p), dma transpose -> sch [c, y, x],
  8 matmuls [64, 512] per unit into ping-pong PSUM regions, Act drains to
  bf16, SWDGE accumulate-DMA into outacc, per-region stores after the last
  tap.

The harness-facing kernel() packs host-side weights (ky-pair / kx-block
layouts, bias vector, edge masks), then runs 8 sequential single-core
launches (the 8-core shard_map path can wedge the device with
NRT_EXEC_UNIT_UNRECOVERABLE, so one core at a time).
"""
import numpy as np
import ml_dtypes
from contextlib import ExitStack

import concourse.bass as bass
from concourse import bacc
import concourse.tile as tile
from concourse import mybir
from concourse.bass_utils import run_bass_kernel_spmd

bf16 = mybir.dt.bfloat16
f32 = mybir.dt.float32
Alu = mybir.AluOpType
Act = mybir.ActivationFunctionType

def mkap(base_ap, extra_off, free_dims):
    """AP over base_ap's tensor: keep its partition dim, custom free dims."""
    p = list(base_ap.ap[0])
    return bass.AP(base_ap.tensor, base_ap.offset + extra_off, [p] + free_dims)


DEBUG = False
B, C, H, W = 8, 64, 128, 128
HW = H * W
NT = 9
XPF = 64 * 128 + 4  # xtc tile free size (pad 2 each side)


def _build():
    nc = bacc.Bacc()
    x_in = nc.dram_tensor("x", [C, HW], bf16, kind="ExternalInput")
    w01a = nc.dram_tensor("w01a", [128, 96], bf16, kind="ExternalInput")
    w23a = nc.dram_tensor("w23a", [128, 96], bf16, kind="ExternalInput")
    w4a = nc.dram_tensor("w4a", [64, 96], bf16, kind="ExternalInput")
    w01b = nc.dram_tensor("w01b", [128, 64], bf16, kind="ExternalInput")
    w23b = nc.dram_tensor("w23b", [128, 64], bf16, kind="ExternalInput")
    w4b = nc.dram_tensor("w4b", [64, 64], bf16, kind="ExternalInput")
    biasA = nc.dram_tensor("biasA", [96, 1], f32, kind="ExternalInput")
    wM = nc.dram_tensor("wM", [NT, 64, 64], bf16, kind="ExternalInput")
    em = nc.dram_tensor("em", [128, 96], bf16, kind="ExternalInput")
    out = nc.dram_tensor("out", [C, HW], f32, kind="ExternalOutput")
    if DEBUG:
        dbg = nc.dram_tensor("dbg", [128, 128 * 32], f32, kind="ExternalOutput")
        dbgX = nc.dram_tensor("dbgX", [128, XPF], f32, kind="ExternalOutput")
        dbgG = nc.dram_tensor("dbgG", [128, 81 * 128], f32, kind="ExternalOutput")
        dbgA = nc.dram_tensor("dbgA", [128, 8192], f32, kind="ExternalOutput")

    with tile.TileContext(nc, pool_alloc_mode="queue") as tc, ExitStack() as ctx:
        pw = ctx.enter_context(tc.tile_pool(name="pw", bufs=1))
        pxtc = ctx.enter_context(tc.tile_pool(name="pxtc", bufs=1))
        xtc = [pxtc.tile([128, XPF], bf16, name=f"xtc{si}") for si in range(5)]
        pgg = ctx.enter_context(tc.tile_pool(name="pgg", bufs=1))
        gg = pgg.tile([128, 81 * 128], bf16, name="gg")
        pofft_cm = tc.tile_pool(name="pofft", bufs=1)
        pofft = pofft_cm.__enter__()
        offt = pofft.tile([128, 128 * 32], bf16, name="offt")

        pxp_cm = tc.tile_pool(name="pxp", bufs=1)
        pxp = pxp_cm.__enter__()

        xpad2 = pxp.tile([128, 16896], bf16, name="xpad2")
        nc.vector.memset(xpad2[0:64, 0:256], 0.0)
        nc.vector.memset(xpad2[0:64, 16640:16896], 0.0)
        nc.vector.memset(xpad2[64:128, 0:128], 0.0)
        nc.vector.memset(xpad2[64:128, 16512:16896], 0.0)
        nc.sync.dma_start(xpad2[0:64, 256:8448], x_in[:, 0:8192])
        nc.scalar.dma_start(xpad2[0:64, 8448:16640], x_in[:, 8192:16384])
        nc.scalar.dma_start(xpad2[64:128, 128:8320], x_in[:, 0:8192])
        nc.gpsimd.dma_start(xpad2[64:128, 8320:16512], x_in[:, 8192:16384])

        w01at = pw.tile([128, 96], bf16, name="w01at")
        w23at = pw.tile([128, 96], bf16, name="w23at")
        w4at = pw.tile([64, 96], bf16, name="w4at")
        w01bt = pw.tile([128, 64], bf16, name="w01bt")
        w23bt = pw.tile([128, 64], bf16, name="w23bt")
        w4bt = pw.tile([64, 64], bf16, name="w4bt")
        nc.sync.dma_start(w01at[:], w01a[:])
        nc.sync.dma_start(w23at[:], w23a[:])
        nc.sync.dma_start(w4at[:], w4a[:])
        nc.sync.dma_start(w01bt[:], w01b[:])
        nc.sync.dma_start(w23bt[:], w23b[:])
        nc.sync.dma_start(w4bt[:], w4b[:])
        biast = pw.tile([96, 1], f32, name="biast")
        nc.sync.dma_start(biast[:], biasA[:])
        wMt = pw.tile([64, NT * 64], bf16, name="wMt")
        nc.sync.dma_start(
            wMt[:].rearrange("c (t o) -> c t o", t=NT),
            wM[:].rearrange("t c o -> c t o"),
        )
        emt = pw.tile([128, 96], bf16, name="emt")
        nc.sync.dma_start(emt[:], em[:])

        # ---------------- shifted transposed-x copies ----------------
        # First 3 reorder copies on DVE (idle during the offset conv, and
        # keeps Act free for the PSUM drains); last 2 on Act (finish during
        # the DVE kx/g-map phase). Tap 0 only needs xtc[0..2].
        with tc.tile_pool(name="pxty", bufs=2) as pxty:
            for si in range(5):
                s = si - 2
                xty = pxty.tile([128, 128, 64], bf16, tag="xty", name="xty")
                nc.sync.dma_start_transpose(
                    xty[:], xpad2[0:64, 256 + s : 256 + s + HW]
                )
                dst = mkap(xtc[si][:], 2, [[128, 64], [1, 128]])
                nc.vector.tensor_copy(out=dst, in_=xty[:].rearrange("x y c -> x c y"))
                nc.vector.memset(xtc[si][:, 0:2], 0.0)
                nc.vector.memset(xtc[si][:, XPF - 2 : XPF], 0.0)

        # ---------------- offset conv ----------------
        # A: kx 0..2 at out partitions kx*32+d; B: kx 3..4 at (kx-3)*32+d.
        # ky pairs {0,1},{2,3} contract 128 via xpad2's row-shifted halves,
        # single ky=4 at 64. The kx column shift is applied by transposing
        # each 32-row block with a shifted source window; edge partitions
        # (wrapped values) are zeroed by small DMAs; DVE sums the 5
        # transposed blocks into offt per y-half, then clamps.
        with tc.tile_pool(name="poff", bufs=1) as poff, tc.tile_pool(
            name="ptk", bufs=2
        ) as ptk, tc.tile_pool(name="psoff", bufs=1, space="PSUM") as psoff:
            HWH = HW // 2
            stOa = poff.tile([96, HWH + 4], bf16, name="stOa")
            stOb = poff.tile([64, HWH + 4], bf16, name="stOb")
            zt = poff.tile([2, 2048], bf16, name="zt")
            nc.vector.memset(zt[:], 0.0)
            nc.vector.memset(stOa[:, 0:2], 0.0)
            nc.vector.memset(stOa[:, HWH + 2 : HWH + 4], 0.0)
            nc.vector.memset(stOb[:, 0:2], 0.0)
            nc.vector.memset(stOb[:, HWH + 2 : HWH + 4], 0.0)
            offtv = offt[:].rearrange("x (y d) -> x y d", d=32)
            for hh in range(2):
                # stripe-major: 4 q x (pA, pB) = 8 PSUM banks per stripe
                for s in range(4 * hh, 4 * hh + 4):
                    pAs = [psoff.tile([96, 512], f32, tag=f"pA{i}", name="pA") for i in range(4)]
                    pBs = [psoff.tile([64, 512], f32, tag=f"pB{i}", name="pB") for i in range(4)]
                    for gi, (wta, wtb, ky0) in enumerate(
                        ((w01at, w01bt, 0), (w23at, w23bt, 2), (w4at, w4bt, 4))
                    ):
                        np_ = 64 if ky0 == 4 else 128
                        for i in range(4):
                            q = s * 4 + i
                            rhs = xpad2[0:np_, q * 512 + ky0 * 128 : q * 512 + ky0 * 128 + 512]
                            nc.tensor.matmul(
                                pAs[i][:], wta[:], rhs, start=(gi == 0), stop=(gi == 2)
                            )
                            nc.tensor.matmul(
                                pBs[i][:], wtb[:], rhs, start=(gi == 0), stop=(gi == 2)
                            )
                    for i in range(4):
                        qq = (s - 4 * hh) * 4 + i
                        nc.scalar.activation(
                            out=stOa[:, 2 + qq * 512 : 2 + (qq + 1) * 512],
                            in_=pAs[i][:], func=Act.Identity, bias=biast[:],
                        )
                        nc.scalar.copy(
                            out=stOb[:, 2 + qq * 512 : 2 + (qq + 1) * 512],
                            in_=pBs[i][:],
                        )
                # shifted block transposes + DVE sum + clamp for this half
                oslc = offt[:, hh * 2048 : (hh + 1) * 2048]
                for kx in (2, 0, 1, 3, 4):
                    co = kx - 2
                    if kx < 3:
                        srct, p0 = stOa, kx * 32
                    else:
                        srct, p0 = stOb, (kx - 3) * 32
                    srcw = srct[p0 : p0 + 32, 2 + co : 2 + co + HWH]
                    if kx == 2:
                        nc.sync.dma_start_transpose(
                            offtv[:, hh * 64 : (hh + 1) * 64, :], srcw
                        )
                        continue
                    tk = ptk.tile([128, 64, 32], bf16, tag="tk", name="tk")
                    nc.sync.dma_start_transpose(tk[:], srcw)
                    nz = abs(co)
                    zr = tk[128 - co : 128, :, :] if co > 0 else tk[0:nz, :, :]
                    nc.scalar.dma_start(
                        zr, zt[0:nz, :].rearrange("p (a b) -> p a b", a=64)
                    )
                    nc.vector.tensor_tensor(
                        out=oslc, in0=oslc,
                        in1=tk[:].rearrange("x y d -> x (y d)"), op=Alu.add,
                    )
                nc.vector.tensor_scalar(
                    out=mkap(offt[:], hh * 2048, [[32, 64], [1, 18]]),
                    in0=mkap(offt[:], hh * 2048, [[32, 64], [1, 18]]),
                    scalar1=1.0, scalar2=-1.0, op0=Alu.min, op1=Alu.max,
                )
            if DEBUG:
                nc.gpsimd.dma_start(dbg[:], offt[:])

        pxp_cm.__exit__(None, None, None)  # free xpad2

        # ---------------- mask / weight maps (built per y-half) ----------------
        with tc.tile_pool(name="pg", bufs=1) as pg:
            mneg = pg.tile([128, 128 * 32], bf16, name="mneg")
            fr = pg.tile([128, 128 * 32], bf16, name="fr")
            omf = pg.tile([128, 128 * 32], bf16, name="omf")
            g = pg.tile([128, 3 * 128 * 32], bf16, name="g")
            t1 = pg.tile([128, 128 * 32], bf16, name="t1")
            gv = g[:].rearrange("x (r y d) -> x r y d", r=3, d=32)
            ggv = gg[:].rearrange(
                "x (ti tj ry rx y) -> x ti tj ry rx y", ti=3, tj=3, ry=3, rx=3
            )
            edge_rows = [
                (0, 0, 0, 2), (0, 1, 0, 1), (1, 0, 0, 1),
                (1, 2, 127, 128), (2, 1, 127, 128), (2, 2, 126, 128),
            ]
            for hh in range(2):
                sl = slice(hh * 2048, (hh + 1) * 2048)
                y0h, y1h = hh * 64, (hh + 1) * 64
                nc.vector.tensor_scalar(
                    out=mneg[:, sl], in0=offt[:, sl], scalar1=0.0, scalar2=None,
                    op0=Alu.is_lt,
                )
                nc.vector.tensor_tensor(
                    out=fr[:, sl], in0=offt[:, sl], in1=mneg[:, sl], op=Alu.add
                )
                nc.vector.tensor_scalar(
                    out=omf[:, sl], in0=fr[:, sl], scalar1=-1.0, scalar2=1.0,
                    op0=Alu.mult, op1=Alu.add,
                )
                for rb in range(3):
                    gsl = slice(rb * 4096 + hh * 2048, rb * 4096 + (hh + 1) * 2048)
                    if rb == 0:
                        nc.vector.tensor_tensor(
                            out=g[:, gsl], in0=mneg[:, sl], in1=omf[:, sl],
                            op=Alu.mult,
                        )
                    elif rb == 1:
                        nc.vector.tensor_tensor(
                            out=t1[:, sl], in0=mneg[:, sl], in1=fr[:, sl],
                            op=Alu.mult,
                        )
                        nc.vector.tensor_tensor(
                            out=g[:, gsl], in0=t1[:, sl], in1=omf[:, sl], op=Alu.add
                        )
                        nc.vector.tensor_tensor(
                            out=g[:, gsl], in0=g[:, gsl],
                            in1=g[:, 0 * 4096 + hh * 2048 : 0 * 4096 + (hh + 1) * 2048],
                            op=Alu.subtract,
                        )
                    else:
                        nc.vector.tensor_tensor(
                            out=g[:, gsl], in0=fr[:, sl], in1=t1[:, sl],
                            op=Alu.subtract,
                        )
                # y-bound masks within this half
                for (ti, ry, ys_, ye) in edge_rows:
                    ys_c, ye_c = max(ys_, y0h), min(ye, y1h)
                    if ys_c >= ye_c:
                        continue
                    for tj in range(3):
                        c0 = 6 * ti + 2 * tj
                        nc.vector.memset(gv[:, ry, ys_c:ye_c, c0 : c0 + 1], 0.0)
                # x-bound masks
                for rx in range(3):
                    blk = gv[:, rx, y0h:y1h, :]
                    em_ap = mkap(emt[:], rx * 32, [[0, 64], [1, 32]])
                    nc.vector.tensor_tensor(out=blk, in0=blk, in1=em_ap, op=Alu.mult)
                # gg[x, (ti tj ry rx y)] = gy * gx for this half
                for ry in range(3):
                    for ti in range(3):
                        gy_ap = mkap(
                            g[:], ry * 4096 + 6 * ti + hh * 2048,
                            [[2, 3], [0, 3], [32, 64]],
                        )
                        gx_ap = mkap(
                            g[:], 6 * ti + 1 + hh * 2048,
                            [[2, 3], [4096, 3], [32, 64]],
                        )
                        nc.vector.tensor_tensor(
                            out=ggv[:, ti, :, ry, :, y0h:y1h], in0=gy_ap,
                            in1=gx_ap, op=Alu.mult,
                        )

        pofft_cm.__exit__(None, None, None)  # free offt
        if DEBUG:
            nc.gpsimd.dma_start(dbgX[:], xtc[2][:])
            nc.gpsimd.dma_start(dbgG[:], gg[:])
        # ---------------- sampling + per-tap finalize ----------------
        # Per tap, two y-halves: acc_h [128, (c, 64y)] stays flat-contiguous
        # so dma_start_transpose reads it directly -> sch_h [c, 64y, 128x].
        pout = ctx.enter_context(tc.tile_pool(name="pout", bufs=1))
        outacc = pout.tile([128, 8192], bf16, name="outacc")
        nc.gpsimd.memset(outacc[:], 0.0)

        pacc = ctx.enter_context(tc.tile_pool(name="pacc", bufs=3))
        ptmp = ctx.enter_context(tc.tile_pool(name="ptmp", bufs=1))
        payc = ctx.enter_context(tc.tile_pool(name="payc", bufs=1))
        psch = ctx.enter_context(tc.tile_pool(name="psch", bufs=1))
        pfin = ctx.enter_context(tc.tile_pool(name="pfin", bufs=2))
        psm = ctx.enter_context(tc.tile_pool(name="psm", bufs=2, space="PSUM"))

        POOL_UNITS = {(3, 0), (6, 0), (0, 1), (6, 1)}
        for h in range(2):
            for t in range(NT):
                ti, tj = t // 3, t % 3
                y0 = h * 64
                ve = nc.gpsimd if (t, h) in POOL_UNITS else nc.vector
                ttag = "tmpP" if (t, h) in POOL_UNITS else "tmp"
                acc = pacc.tile([128, 4096], bf16, tag="acc", name="acc")
                first = True
                for ry in range(3):
                    ro = ti - 2 + ry
                    for rx in range(3):
                        co = tj - 2 + rx
                        m = t * 9 + ry * 3 + rx
                        xs_t = xtc[co + 2]
                        in0 = mkap(xs_t[:], 2 + ro + y0, [[128, 64], [1, 64]])
                        in1 = mkap(gg[:], m * 128 + y0, [[0, 64], [1, 64]])
                        if first:
                            o_ap = mkap(acc[:], 0, [[64, 64], [1, 64]])
                            ve.tensor_tensor(
                                out=o_ap, in0=in0, in1=in1, op=Alu.mult
                            )
                            first = False
                        else:
                            tmp = ptmp.tile([128, 4096], bf16, tag=ttag, name="tmp")
                            ve.tensor_tensor(
                                out=mkap(tmp[:], 0, [[64, 64], [1, 64]]),
                                in0=in0, in1=in1, op=Alu.mult,
                            )
                            ve.tensor_tensor(
                                out=acc[:], in0=acc[:], in1=tmp[:], op=Alu.add
                            )

                # ayc pitch-128 half-filled: HW 3D transpose reads f = m*128+p
                ayc = payc.tile([128, 8192], bf16, tag="ayc", name="ayc")
                nc.scalar.copy(
                    out=mkap(ayc[:], 0, [[128, 64], [1, 64]]),
                    in_=mkap(acc[:], 0, [[1, 64], [64, 64]]),
                )
                sch = psch.tile([64, 64, 128], bf16, tag="sch", name="sch")
                nc.sync.dma_start_transpose(sch[:], ayc[:])
                schv = sch[:].rearrange("c y x -> c (y x)")
                for rr in range(4):
                    r = h * 4 + rr
                    pm = psm.tile([64, 2048], f32, tag="pm", name="pm")
                    for j in range(4):
                        nc.tensor.matmul(
                            pm[:, j * 512 : (j + 1) * 512],
                            wMt[:, t * 64 : (t + 1) * 64],
                            schv[:, rr * 2048 + j * 512 : rr * 2048 + (j + 1) * 512],
                            start=True, stop=True,
                        )
                    tpd = pfin.tile([64, 2048], bf16, tag="tpd", name="tpd")
                    nc.scalar.copy(tpd[:], pm[:])
                    half, off = divmod(r * 2048, 8192)
                    oslice = outacc[64 * half : 64 * half + 64, off : off + 2048]
                    nc.gpsimd.dma_start(out=oslice, in_=tpd[:], accum_op=Alu.add)
                    if t == NT - 1:
                        nc.gpsimd.dma_start(
                            out[:, r * 2048 : (r + 1) * 2048], oslice
                        )

        if DEBUG:
            nc.gpsimd.dma_start(dbgA[:], outacc[:])

    nc.compile()
    return nc


_NC = None


def _get_nc():
    global _NC
    if _NC is None:
        _NC = _build()
    return _NC


def _pack_inputs(x, weights, offset_w, offset_b):
    x = np.asarray(x, dtype=np.float32)
    weights = np.asarray(weights, dtype=np.float32)
    offset_w = np.asarray(offset_w, dtype=np.float32)
    offset_b = np.asarray(offset_b, dtype=np.float32)

    # offset conv weights: A-side kx 0..2 at col kx*32+d, B-side kx 3..4 at
    # (kx-3)*32+d; row halves = ky pair
    w01a = np.zeros((128, 96), np.float32)
    w23a = np.zeros((128, 96), np.float32)
    w4a = np.zeros((64, 96), np.float32)
    w01b = np.zeros((128, 64), np.float32)
    w23b = np.zeros((128, 64), np.float32)
    w4b = np.zeros((64, 64), np.float32)
    for kx in range(3):
        w01a[0:64, kx * 32 : kx * 32 + 18] = offset_w[:, :, 0, kx].T
        w01a[64:128, kx * 32 : kx * 32 + 18] = offset_w[:, :, 1, kx].T
        w23a[0:64, kx * 32 : kx * 32 + 18] = offset_w[:, :, 2, kx].T
        w23a[64:128, kx * 32 : kx * 32 + 18] = offset_w[:, :, 3, kx].T
        w4a[0:64, kx * 32 : kx * 32 + 18] = offset_w[:, :, 4, kx].T
    for kx in (3, 4):
        cb = (kx - 3) * 32
        w01b[0:64, cb : cb + 18] = offset_w[:, :, 0, kx].T
        w01b[64:128, cb : cb + 18] = offset_w[:, :, 1, kx].T
        w23b[0:64, cb : cb + 18] = offset_w[:, :, 2, kx].T
        w23b[64:128, cb : cb + 18] = offset_w[:, :, 3, kx].T
        w4b[0:64, cb : cb + 18] = offset_w[:, :, 4, kx].T
    biasA = np.zeros((96, 1), np.float32)
    biasA[64:82, 0] = offset_b

    wM = weights.reshape(C, C, 9).transpose(2, 1, 0).copy()
    em = np.ones((128, 96), np.float32)
    xs_ = np.arange(128)
    for rx in range(3):
        for ti in range(3):
            for tj in range(3):
                co = tj - 2 + rx
                em[:, rx * 32 + 2 * (3 * ti + tj) + 1] = (
                    (xs_ + co >= 0) & (xs_ + co < 128)
                )

    cast = lambda a: np.ascontiguousarray(a).astype(ml_dtypes.bfloat16)
    in_maps = [
        {
            "x": cast(x[b].reshape(C, HW)),
            "w01a": cast(w01a),
            "w23a": cast(w23a),
            "w4a": cast(w4a),
            "w01b": cast(w01b),
            "w23b": cast(w23b),
            "w4b": cast(w4b),
            "biasA": np.ascontiguousarray(biasA),
            "wM": cast(wM),
            "em": cast(em),
        }
        for b in range(B)
    ]
    return in_maps


def kernel(x, weights, offset_w, offset_b):
    in_maps = _pack_inputs(x, weights, offset_w, offset_b)
    nc = _get_nc()
    # Per-sample sequential execution. The 8-core shard_map path triggers an
    # engine hang (NRT_EXEC_UNIT_UNRECOVERABLE) that wedges the device for
    # the rest of the process, so run one core at a time.
    outs = []
    for b in range(B):
        r1 = run_bass_kernel_spmd(nc, [in_maps[b]], [0])
        outs.append(np.asarray(r1.results[0]["out"]))
    return np.stack([o.reshape(C, H, W) for o in outs]).astype(np.float32)
